# revision 1
# baseline (speedup 1.0000x reference)
"""Trainium2 Bass kernel for nn_Attn_128849019074 (sparse_attention).

reference:
    energy = einsum("lbd,ed->lbe", enc, W) + b        # [L,B,D] huge matmul
    scores = einsum("lbd,bd->lb", energy, hidden)     # [L,B]
    out    = log_softmax(scores, axis=1)[None, None]  # [1,1,L,B]

Algebraic rewrite (linearity):
    scores[l,b] = enc[l,b,:] . v[b,:] + c[b]
    with v = hidden @ W   ([B,D], tiny)  and  c = hidden @ b  ([B]).

This turns a 137-GMAC matmul into a single streaming pass over
encoder_outputs (268 MB) -> memory bound.

Distribution: shard over L (128 timesteps per core).  The dim=1
log-softmax is over B=32, which is fully local per (l) row -> no
collective needed for the softmax.  v is computed column-sharded
(each core does 256 of the 2048 columns on the PE) and AllGather'd.

Per-core dataflow:
  - enc chunk arrives as 8 contiguous 4-MB DMAs into SBUF tiles
    [128 partitions = (4 l's x 32 b's), 4, 2048].
  - one fused DVE tensor_tensor_reduce per l-group:
        prod = enc_tile * vbig ; scores[:, col] = sum_d prod
    where vbig[p, d] = v[p % 32, d] (v replicated 4x on partitions).
  - scores [128, 32] -> +c -> PE transpose -> [32 tiles, 128=(4a x 32b)]
    -> segmented (per-32) max / exp-accum / ln -> out rows l = 4t + a.
"""

import os
import sys

sys.path.insert(0, "/opt/trn_rl_repo")

import numpy as np

L = 1024
B = 32
D = 2048
NCORES = 8
L_LOC = L // NCORES          # 128 timesteps per core
D_SH = D // NCORES           # 256 v-columns computed per core
G = 4                        # l-groups per enc tile
N_TILES = L_LOC // (4 * G)   # 8 enc DMAs of [128, G, D] (4 MB each)
N_COLS = L_LOC // 4          # 32 score columns (one per l-group)

_CACHE: dict = {}
last_results = None          # BassKernelResults from the most recent run


def _split_drain_waits(nc):
    """Walrus rejects Drain instructions carrying many sync waits ("Too many
    sync wait commands").  Tile's kernel-tail drain waits on every live
    semaphore lane at once; split it into a chain of single-wait drains."""
    import concourse.mybir as mybir

    for bb in nc.main_func.blocks:
        idx = 0
        while idx < len(bb.instructions):
            inst = bb.instructions[idx]
            if (
                isinstance(inst, mybir.InstDrain)
                and inst.sync_info is not None
                and len(inst.sync_info.on_wait or []) > 1
            ):
                waits = list(inst.sync_info.on_wait)
                spill, keep = waits[:-1], waits[-1:]
                new_insts = []
                for j, w in enumerate(spill):
                    x = mybir.InstDrain(name=f"{inst.name}_w{j}", ins=[], outs=[])
                    x.engine = inst.engine
                    x.sync_info = mybir.SyncInfo(on_wait=[w], on_update=[])
                    x.debug = inst.debug
                    nc.register_instruction(x)
                    new_insts.append(x)
                inst.sync_info = mybir.SyncInfo(
                    on_wait=keep, on_update=list(inst.sync_info.on_update or [])
                )
                bb.instructions[idx:idx] = new_insts
                idx += len(new_insts)
            idx += 1


def build_program():
    """Build (once) the SPMD Bass program shared by all 8 cores."""
    if "nc" in _CACHE:
        return _CACHE["nc"]

    import concourse.bacc as bacc
    import concourse.mybir as mybir
    import concourse.tile as tile

    f32 = mybir.dt.float32
    Alu = mybir.AluOpType
    Act = mybir.ActivationFunctionType

    nc = bacc.Bacc(
        "TRN2", target_bir_lowering=False, debug=False, num_devices=NCORES
    )

    enc = nc.dram_tensor("enc", [L_LOC * B, D], f32, kind="ExternalInput").ap()
    # hbtt[p, 33c + j] = [hidden; b]^T[128c + p, j] — pre-tiled on the host so
    # the DMA is one contiguous run per partition.
    hbtt = nc.dram_tensor(
        "hbtt", [128, 16 * (B + 1)], f32, kind="ExternalInput"
    ).ap()
    wfull = nc.dram_tensor("wfull", [D, D], f32, kind="ExternalInput").ap()
    out = nc.dram_tensor("out", [L_LOC, B], f32, kind="ExternalOutput").ap()
    ident = nc.inline_tensor(np.eye(128, dtype=np.float32), "ident").ap()
    # repl[b, p] = 1 if p % 32 == b — PE-side partition replication matrix
    repl = nc.inline_tensor(
        np.ascontiguousarray(np.tile(np.eye(B, dtype=np.float32), (1, 4))),
        "repl",
    ).ap()

    with tile.TileContext(nc) as tc:
        with (
            tc.tile_pool(name="pers", bufs=1) as pers,
            tc.tile_pool(name="encp", bufs=3) as encp,
            tc.tile_pool(name="wp", bufs=6) as wp,
            tc.tile_pool(name="prodp", bufs=2) as prodp,
            tc.tile_pool(name="psp", bufs=1, space="PSUM") as psp,
        ):
            # ---------- phase 1: v = hidden @ W and c = hidden @ b on PE ----
            # No collectives: every core reads the full (replicated) W.  The
            # NEFF-start barrier + collective trigger latency (~75 us measured)
            # costs far more than the extra 16 MB W read (~45 us).
            hbt_sb = pers.tile([128, 16 * (B + 1)], f32)
            nc.sync.dma_start(hbt_sb[:, :], hbtt[:, :])
            ident_sb = pers.tile([128, 128], f32)
            nc.sync.dma_start(ident_sb[:, :], ident[:, :])
            repl_sb = pers.tile([B, 128], f32)
            nc.sync.dma_start(repl_sb[:, :], repl[:, :])

            # PE warm-up: ~4.5 us of back-to-back matmuls flips the HAM clock
            # gate from 1.2 to 2.4 GHz before the W-chunk matmuls begin.
            warm_ps = psp.tile([128, 512], f32)
            for i in range(10):
                nc.tensor.matmul(
                    warm_ps[:, :], hbt_sb[:, 0:128], hbt_sb[:, 0:512]
                )
            warm_junk = pers.tile([1, 1], f32)
            nc.vector.tensor_copy(warm_junk[:, :], warm_ps[0:1, 0:1])

            v_ps = psp.tile([B, D], f32, tag="big")
            c_ps = psp.tile([B, 1], f32)
            last_w_dma = None
            for c in range(16):
                wc = wp.tile([128, D], f32, tag="wc")
                last_w_dma = nc.sync.dma_start(
                    wc[:, :], wfull[128 * c : 128 * (c + 1), :]
                )
                lhs = hbt_sb[:, (B + 1) * c : (B + 1) * c + B]
                for n in range(4):
                    nc.tensor.matmul(
                        v_ps[:, 512 * n : 512 * (n + 1)],
                        lhs,
                        wc[:, 512 * n : 512 * (n + 1)],
                        start=(c == 0),
                        stop=(c == 15),
                    )
            for c in range(16):
                lhs = hbt_sb[:, (B + 1) * c : (B + 1) * c + B]
                rhs = hbt_sb[:, (B + 1) * c + B : (B + 1) * (c + 1)]
                nc.tensor.matmul(
                    c_ps[:, :], lhs, rhs, start=(c == 0), stop=(c == 15)
                )

            # ---------- phase 2: replicate v/c 4x across partitions via PE --
            # vbig[32a + b, d] = v[b, d]; staged through rows 0-31 of vbig.
            vbig = pers.tile([128, D], f32)
            nc.vector.tensor_copy(vbig[0:B, :], v_ps[:, :])
            vbig_ps = psp.tile([128, D], f32, tag="big")
            for n in range(4):
                nc.tensor.matmul(
                    vbig_ps[:, 512 * n : 512 * (n + 1)],
                    repl_sb[:, :],
                    vbig[0:B, 512 * n : 512 * (n + 1)],
                )
            nc.vector.tensor_copy(vbig[:, :], vbig_ps[:, :])
            cbig = pers.tile([128, 1], f32)
            nc.vector.tensor_copy(cbig[0:B, :], c_ps[:, :])
            cbig_ps = psp.tile([128, 1], f32, tag="small_ps")
            nc.tensor.matmul(cbig_ps[:, :], repl_sb[:, :], cbig[0:B, :])
            nc.vector.tensor_copy(cbig[:, :], cbig_ps[:, :])

            # ---------- phase 3: stream enc, dot with v ----------
            # DVE does the elementwise multiply; the (otherwise idle) ACT
            # engine does the free-axis reduction via activation accum_out.
            scores = pers.tile([128, N_COLS], f32)
            ascr = pers.tile([128, D], f32)  # ACT dummy out
            for t in range(N_TILES):
                et = encp.tile([128, G, D], f32, tag="et")
                enc_dma = nc.sync.dma_start(
                    et[:, :, :],
                    enc[128 * G * t : 128 * G * (t + 1), :].rearrange(
                        "(g p) d -> p g d", p=128
                    ),
                )
                # Keep the HWDGE FIFO ring W-first: v (and thus vbig) must be
                # ready early so the DVE can drain enc tiles as they land.
                tile.add_dep_helper(
                    enc_dma.ins,
                    last_w_dma.ins,
                    sync=False,
                    reason="enc stream after W (W-first DMA ordering)",
                )
                for g in range(G):
                    pt = prodp.tile([128, D], f32, tag="pt")
                    nc.vector.tensor_mul(pt[:, :], et[:, g, :], vbig[:, :])
                    nc.scalar.activation(
                        ascr[:, :],
                        pt[:, :],
                        Act.Copy,
                        accum_out=scores[:, G * t + g : G * t + g + 1],
                    )

            # ---------- phase 4: + c, transpose, log-softmax over b ----------
            sc2 = pers.tile([128, N_COLS], f32)
            nc.vector.tensor_scalar_add(sc2[:, :], scores[:, :], cbig[:, 0:1])
            scT_ps = psp.tile([N_COLS, 128], f32, tag="small_ps")
            nc.tensor.transpose(scT_ps[:, :], sc2[:, :], ident_sb[:, :])
            tsb = pers.tile([N_COLS, 128], f32)
            nc.vector.tensor_copy(tsb[:, :], scT_ps[:, :])
            m4 = pers.tile([N_COLS, 4], f32)
            nc.vector.tensor_reduce(
                m4[:, :],
                tsb.rearrange("p (a b) -> p a b", b=B),
                axis=mybir.AxisListType.X,
                op=Alu.max,
            )
            sm = pers.tile([N_COLS, 128], f32)
            s4 = pers.tile([N_COLS, 4], f32)
            es = pers.tile([N_COLS, 128], f32)
            for a in range(4):
                nc.vector.tensor_scalar_sub(
                    sm[:, B * a : B * (a + 1)],
                    tsb[:, B * a : B * (a + 1)],
                    m4[:, a : a + 1],
                )
            for a in range(4):
                nc.scalar.activation(
                    es[:, B * a : B * (a + 1)],
                    sm[:, B * a : B * (a + 1)],
                    Act.Exp,
                    accum_out=s4[:, a : a + 1],
                )
            ls4 = pers.tile([N_COLS, 4], f32)
            nc.scalar.activation(ls4[:, :], s4[:, :], Act.Ln)
            o = pers.tile([N_COLS, 128], f32)
            for a in range(4):
                nc.vector.tensor_scalar_sub(
                    o[:, B * a : B * (a + 1)],
                    sm[:, B * a : B * (a + 1)],
                    ls4[:, a : a + 1],
                )
            # out[4t + a, b] = o[t, 32a + b]
            out_r = out.rearrange("(t a) b -> t a b", a=4)
            for a in range(4):
                nc.sync.dma_start(out_r[:, a, :], o[:, B * a : B * (a + 1)])

    nc.compile()
    _split_drain_waits(nc)
    _CACHE["nc"] = nc
    return nc


def make_in_maps(hidden, encoder_outputs, W, b):
    hidden = np.ascontiguousarray(np.asarray(hidden, dtype=np.float32))
    enc = np.ascontiguousarray(np.asarray(encoder_outputs, dtype=np.float32))
    W_ = np.ascontiguousarray(np.asarray(W, dtype=np.float32))
    b_ = np.ascontiguousarray(np.asarray(b, dtype=np.float32))
    hb = np.concatenate([hidden, b_[None, :]], axis=0)  # [33, D]
    # hbtt[p, 33c + j] = hb[j, 128c + p] — the SBUF tile layout, host-built
    hbtt = np.ascontiguousarray(
        hb.T.reshape(16, 128, B + 1).transpose(1, 0, 2).reshape(128, 16 * (B + 1))
    )
    in_maps = []
    for k in range(NCORES):
        in_maps.append(
            {
                "enc": np.ascontiguousarray(
                    enc[k * L_LOC : (k + 1) * L_LOC].reshape(L_LOC * B, D)
                ),
                "hbtt": hbtt,
                "wfull": W_,
            }
        )
    return in_maps


def kernel(hidden, encoder_outputs, W, b):
    """Full inputs in, full [1, 1, L, B] output out; runs on 8 NeuronCores."""
    global last_results
    from concourse.bass_utils import run_bass_kernel_spmd

    nc = build_program()
    in_maps = make_in_maps(hidden, encoder_outputs, W, b)
    res = run_bass_kernel_spmd(
        nc,
        in_maps,
        list(range(NCORES)),
        trace=bool(os.environ.get("KERNEL_TRACE")),
    )
    last_results = res
    chunks = [res.results[k]["out"] for k in range(NCORES)]
    full = np.concatenate(chunks, axis=0).reshape(1, 1, L, B)
    return full.astype(np.float32)



# revision 5
# speedup vs baseline: 1.3507x; 1.3507x over previous
"""Trainium2 Bass kernel for nn_Attn_128849019074 (sparse_attention).

reference:
    energy = einsum("lbd,ed->lbe", enc, W) + b        # [L,B,D] huge matmul
    scores = einsum("lbd,bd->lb", energy, hidden)     # [L,B]
    out    = log_softmax(scores, axis=1)[None, None]  # [1,1,L,B]

Algebraic rewrite (linearity):
    scores[l,b] = enc[l,b,:] . v[b,:] + c[b]
    with v = hidden @ W   ([B,D], tiny)  and  c = hidden @ b  ([B]).

This turns a 137-GMAC matmul into a single streaming pass over
encoder_outputs -> memory bound.  All streamed operands are cast to
bf16 on the host (tolerance is 2e-2; bf16 end-to-end sims at 3.4e-3),
halving HBM traffic to 17 MB/core and doubling DVE throughput.

Distribution: shard over L (128 timesteps per core).  The dim=1
log-softmax is over B=32, which is fully local per (l) row -> no
collective needed for the softmax.

Per-core dataflow:
  - W (bf16, 8 MB) DMA'd first; v = hidden @ W accumulated on the PE
    in fp32 PSUM; v replicated 4x across partitions via a PE matmul
    with a 0/1 matrix, stored as bf16 vbig[p, d] = v[p % 32, d].
  - enc chunk arrives as 8 contiguous 2-MB DMAs into SBUF tiles
    [128 partitions = (4 l's x 32 b's), 4, 2048] bf16.
  - one fused DVE tensor_tensor_reduce per l-group:
        prod = enc_tile * vbig ; scores[:, col] = sum_d prod
    (all operands bf16/packed/SBUF -> high-rate DVE mode; accum fp32).
  - scores [128, 32] -> +c -> PE transpose -> [32 tiles, 128=(4a x 32b)]
    -> segmented (per-32) max / exp-accum / ln -> out rows l = 4t + a.
"""

import os
import sys

sys.path.insert(0, "/opt/trn_rl_repo")

import numpy as np

L = 1024
B = 32
D = 2048
NCORES = 8
L_LOC = L // NCORES          # 128 timesteps per core
G = 4                        # l-groups per enc tile
N_TILES = L_LOC // (4 * G)   # 8 enc DMAs of [128, G, D] (2 MB each)
N_COLS = L_LOC // 4          # 32 score columns (one per l-group)

_CACHE: dict = {}
last_results = None          # BassKernelResults from the most recent run

USE_BF16 = os.environ.get("KERNEL_BF16", "1") == "1"
USE_TTR = os.environ.get("KERNEL_TTR", "1") == "1"


def _split_drain_waits(nc):
    """Walrus rejects Drain instructions carrying many sync waits ("Too many
    sync wait commands").  Tile's kernel-tail drain waits on every live
    semaphore lane at once; split it into a chain of single-wait drains."""
    import concourse.mybir as mybir

    for bb in nc.main_func.blocks:
        idx = 0
        while idx < len(bb.instructions):
            inst = bb.instructions[idx]
            if (
                isinstance(inst, mybir.InstDrain)
                and inst.sync_info is not None
                and len(inst.sync_info.on_wait or []) > 1
            ):
                waits = list(inst.sync_info.on_wait)
                spill, keep = waits[:-1], waits[-1:]
                new_insts = []
                for j, w in enumerate(spill):
                    x = mybir.InstDrain(name=f"{inst.name}_w{j}", ins=[], outs=[])
                    x.engine = inst.engine
                    x.sync_info = mybir.SyncInfo(on_wait=[w], on_update=[])
                    x.debug = inst.debug
                    nc.register_instruction(x)
                    new_insts.append(x)
                inst.sync_info = mybir.SyncInfo(
                    on_wait=keep, on_update=list(inst.sync_info.on_update or [])
                )
                bb.instructions[idx:idx] = new_insts
                idx += len(new_insts)
            idx += 1


def build_program():
    """Build (once) the SPMD Bass program shared by all 8 cores."""
    if "nc" in _CACHE:
        return _CACHE["nc"]

    import concourse.bacc as bacc
    import concourse.mybir as mybir
    import concourse.tile as tile

    f32 = mybir.dt.float32
    bf16 = mybir.dt.bfloat16
    dt16 = bf16 if USE_BF16 else f32
    Alu = mybir.AluOpType
    Act = mybir.ActivationFunctionType

    nc = bacc.Bacc(
        "TRN2", target_bir_lowering=False, debug=False, num_devices=NCORES
    )

    enc = nc.dram_tensor("enc", [L_LOC * B, D], dt16, kind="ExternalInput").ap()
    # hbtt[p, 33c + j] = [hidden; b]^T[128c + p, j] — pre-tiled on the host so
    # the DMA is one contiguous run per partition.
    hbtt = nc.dram_tensor(
        "hbtt", [128, 16 * (B + 1)], dt16, kind="ExternalInput"
    ).ap()
    wfull = nc.dram_tensor("wfull", [D, D], dt16, kind="ExternalInput").ap()
    out = nc.dram_tensor("out", [L_LOC, B], f32, kind="ExternalOutput").ap()
    ident = nc.inline_tensor(np.eye(128, dtype=np.float32), "ident").ap()
    # repl[b, p] = 1 if p % 32 == b — PE-side partition replication matrix
    import ml_dtypes

    repl_np = np.ascontiguousarray(np.tile(np.eye(B, dtype=np.float32), (1, 4)))
    if USE_BF16:
        repl_np = repl_np.astype(ml_dtypes.bfloat16)
    repl = nc.inline_tensor(repl_np, "repl").ap()

    with tile.TileContext(nc) as tc:
        with (
            tc.tile_pool(name="pers", bufs=1) as pers,
            tc.tile_pool(name="encp", bufs=4 if USE_BF16 else 3) as encp,
            tc.tile_pool(name="wp", bufs=6) as wp,
            tc.tile_pool(name="prodp", bufs=2) as prodp,
            tc.tile_pool(name="psp", bufs=1, space="PSUM") as psp,
        ):
            # ---------- phase 1: v = hidden @ W and c = hidden @ b on PE ----
            # No collectives: every core reads the full (replicated) bf16 W.
            hbt_sb = pers.tile([128, 16 * (B + 1)], dt16)
            nc.sync.dma_start(hbt_sb[:, :], hbtt[:, :])
            ident_sb = pers.tile([128, 128], f32)
            nc.sync.dma_start(ident_sb[:, :], ident[:, :])
            repl_sb = pers.tile([B, 128], dt16)
            nc.sync.dma_start(repl_sb[:, :], repl[:, :])

            # PE warm-up: ~4.5 us of back-to-back matmuls flips the HAM clock
            # gate from 1.2 to 2.4 GHz before the W-chunk matmuls begin.
            warm_ps = psp.tile([128, 512], f32)
            for i in range(10):
                nc.tensor.matmul(
                    warm_ps[:, :], hbt_sb[:, 0:128], hbt_sb[:, 0:512]
                )
            warm_junk = pers.tile([1, 1], f32)
            nc.vector.tensor_copy(warm_junk[:, :], warm_ps[0:1, 0:1])

            v_ps = psp.tile([B, D], f32, tag="big")
            c_ps = psp.tile([B, 1], f32)
            last_w_dma = None
            for c in range(16):
                wc = wp.tile([128, D], dt16, tag="wc")
                last_w_dma = nc.sync.dma_start(
                    wc[:, :], wfull[128 * c : 128 * (c + 1), :]
                )
                lhs = hbt_sb[:, (B + 1) * c : (B + 1) * c + B]
                for n in range(4):
                    nc.tensor.matmul(
                        v_ps[:, 512 * n : 512 * (n + 1)],
                        lhs,
                        wc[:, 512 * n : 512 * (n + 1)],
                        start=(c == 0),
                        stop=(c == 15),
                    )
            for c in range(16):
                lhs = hbt_sb[:, (B + 1) * c : (B + 1) * c + B]
                rhs = hbt_sb[:, (B + 1) * c + B : (B + 1) * (c + 1)]
                nc.tensor.matmul(
                    c_ps[:, :], lhs, rhs, start=(c == 0), stop=(c == 15)
                )

            # ---------- phase 2: replicate v/c 4x across partitions via PE --
            # vbig[32a + b, d] = v[b, d] in bf16 (exact: values already bf16
            # after the vsmall cast; repl is 0/1).
            vsmall = pers.tile([B, D], dt16)
            nc.vector.tensor_copy(vsmall[:, :], v_ps[:, :])
            vbig = pers.tile([128, D], dt16)
            vbig_ps = psp.tile([128, D], f32, tag="big")
            for n in range(4):
                nc.tensor.matmul(
                    vbig_ps[:, 512 * n : 512 * (n + 1)],
                    repl_sb[:, :],
                    vsmall[:, 512 * n : 512 * (n + 1)],
                )
                nc.vector.tensor_copy(
                    vbig[:, 512 * n : 512 * (n + 1)],
                    vbig_ps[:, 512 * n : 512 * (n + 1)],
                )
            csmall = pers.tile([B, 1], dt16)
            nc.vector.tensor_copy(csmall[:, :], c_ps[:, :])
            cbig = pers.tile([128, 1], f32)
            cbig_ps = psp.tile([128, 1], f32, tag="small_ps")
            nc.tensor.matmul(cbig_ps[:, :], repl_sb[:, :], csmall[:, :])
            nc.vector.tensor_copy(cbig[:, :], cbig_ps[:, :])

            # ---------- phase 3: stream enc, dot with v on the DVE ----------
            # One fused tensor_tensor_reduce per l-group: prod (bf16, thrown
            # away) and the fp32 per-partition sum land in one instruction.
            scores = pers.tile([128, N_COLS], f32)
            ascr = pers.tile([128, D], f32)  # ACT dummy out (non-TTR path)
            for t in range(N_TILES):
                et = encp.tile([128, G, D], dt16, tag="et")
                enc_dma = nc.sync.dma_start(
                    et[:, :, :],
                    enc[128 * G * t : 128 * G * (t + 1), :].rearrange(
                        "(g p) d -> p g d", p=128
                    ),
                )
                # Keep the HWDGE FIFO ring W-first: v (and thus vbig) must be
                # ready early so the DVE can drain enc tiles as they land.
                tile.add_dep_helper(
                    enc_dma.ins,
                    last_w_dma.ins,
                    sync=False,
                    reason="enc stream after W (W-first DMA ordering)",
                )
                for g in range(G):
                    pt = prodp.tile([128, D], dt16, tag="pt")
                    if USE_TTR:
                        nc.vector.tensor_tensor_reduce(
                            pt[:, :],
                            et[:, g, :],
                            vbig[:, :],
                            scale=1.0,
                            scalar=0.0,
                            op0=Alu.mult,
                            op1=Alu.add,
                            accum_out=scores[:, G * t + g : G * t + g + 1],
                        )
                    else:
                        nc.vector.tensor_mul(pt[:, :], et[:, g, :], vbig[:, :])
                        nc.scalar.activation(
                            ascr[:, :],
                            pt[:, :],
                            Act.Copy,
                            accum_out=scores[:, G * t + g : G * t + g + 1],
                        )

            # ---------- phase 4: + c, transpose, log-softmax over b ----------
            sc2 = pers.tile([128, N_COLS], f32)
            nc.vector.tensor_scalar_add(sc2[:, :], scores[:, :], cbig[:, 0:1])
            scT_ps = psp.tile([N_COLS, 128], f32, tag="small_ps")
            nc.tensor.transpose(scT_ps[:, :], sc2[:, :], ident_sb[:, :])
            tsb = pers.tile([N_COLS, 128], f32)
            nc.vector.tensor_copy(tsb[:, :], scT_ps[:, :])
            m4 = pers.tile([N_COLS, 4], f32)
            nc.vector.tensor_reduce(
                m4[:, :],
                tsb.rearrange("p (a b) -> p a b", b=B),
                axis=mybir.AxisListType.X,
                op=Alu.max,
            )
            sm = pers.tile([N_COLS, 128], f32)
            s4 = pers.tile([N_COLS, 4], f32)
            es = pers.tile([N_COLS, 128], f32)
            for a in range(4):
                nc.vector.tensor_scalar_sub(
                    sm[:, B * a : B * (a + 1)],
                    tsb[:, B * a : B * (a + 1)],
                    m4[:, a : a + 1],
                )
            for a in range(4):
                nc.scalar.activation(
                    es[:, B * a : B * (a + 1)],
                    sm[:, B * a : B * (a + 1)],
                    Act.Exp,
                    accum_out=s4[:, a : a + 1],
                )
            ls4 = pers.tile([N_COLS, 4], f32)
            nc.scalar.activation(ls4[:, :], s4[:, :], Act.Ln)
            o = pers.tile([N_COLS, 128], f32)
            for a in range(4):
                nc.vector.tensor_scalar_sub(
                    o[:, B * a : B * (a + 1)],
                    sm[:, B * a : B * (a + 1)],
                    ls4[:, a : a + 1],
                )
            # out[4t + a, b] = o[t, 32a + b]
            out_r = out.rearrange("(t a) b -> t a b", a=4)
            for a in range(4):
                nc.sync.dma_start(out_r[:, a, :], o[:, B * a : B * (a + 1)])

    nc.compile()
    _split_drain_waits(nc)
    _CACHE["nc"] = nc
    return nc


def make_in_maps(hidden, encoder_outputs, W, b):
    import ml_dtypes

    bf = ml_dtypes.bfloat16 if USE_BF16 else np.float32
    hidden = np.asarray(hidden, dtype=np.float32)
    enc = np.asarray(encoder_outputs, dtype=np.float32)
    W_ = np.ascontiguousarray(np.asarray(W, dtype=np.float32).astype(bf))
    b_ = np.asarray(b, dtype=np.float32)
    hb = np.concatenate([hidden, b_[None, :]], axis=0)  # [33, D]
    # hbtt[p, 33c + j] = hb[j, 128c + p] — the SBUF tile layout, host-built
    hbtt = np.ascontiguousarray(
        hb.T.reshape(16, 128, B + 1).transpose(1, 0, 2).reshape(128, 16 * (B + 1))
    ).astype(bf)
    enc_bf = enc.reshape(NCORES, L_LOC * B, D).astype(bf)
    in_maps = []
    for k in range(NCORES):
        in_maps.append(
            {
                "enc": np.ascontiguousarray(enc_bf[k]),
                "hbtt": hbtt,
                "wfull": W_,
            }
        )
    return in_maps


def kernel(hidden, encoder_outputs, W, b):
    """Full inputs in, full [1, 1, L, B] output out; runs on 8 NeuronCores."""
    global last_results
    from concourse.bass_utils import run_bass_kernel_spmd

    nc = build_program()
    in_maps = make_in_maps(hidden, encoder_outputs, W, b)
    res = run_bass_kernel_spmd(
        nc,
        in_maps,
        list(range(NCORES)),
        trace=bool(os.environ.get("KERNEL_TRACE")),
    )
    last_results = res
    chunks = [res.results[k]["out"] for k in range(NCORES)]
    full = np.concatenate(chunks, axis=0).reshape(1, 1, L, B)
    return full.astype(np.float32)


# revision 10
# speedup vs baseline: 1.9406x; 1.4368x over previous
"""Trainium2 Bass kernel for nn_Attn_128849019074 (sparse_attention).

reference:
    energy = einsum("lbd,ed->lbe", enc, W) + b        # [L,B,D] huge matmul
    scores = einsum("lbd,bd->lb", energy, hidden)     # [L,B]
    out    = log_softmax(scores, axis=1)[None, None]  # [1,1,L,B]

Algebraic rewrite (linearity):
    scores[l,b] = enc[l,b,:] . v[b,:] + c[b]
    with v = hidden @ W   ([B,D], tiny)  and  c = hidden @ b  ([B]).

This turns a 137-GMAC matmul into a single streaming pass over
encoder_outputs -> memory bound.  All streamed operands are cast to
bf16 on the host (tolerance is 2e-2; bf16 end-to-end sims at ~3e-3),
halving HBM traffic to ~24 MB/core.

Distribution: shard over L (128 timesteps per core).  The dim=1
log-softmax is over B=32, fully local per l row -> no collectives.

Per-core dataflow (everything on the PE; DVE/ACT nearly idle):
  - enc host-transposed to [D, B, L_LOC] so the contraction dim d lies
    on SBUF partitions; streamed as 8 x 2-MB DMAs [128, 2, B*L_LOC].
  - W host-packed column-blocks wt[t][p, 128c+j] = W[128c+p, 128t+j];
    vT[d,b] = sum_e W[e,d] h[b,e] accumulated on the PE -> [128,16,32].
  - score matmuls: for d-chunk t, batch b:
        sc[l, b] += encT[128t:128(t+1), b, :]^T @ vT[128t:128(t+1), b]
    i.e. lhsT = the enc block [128, 128] (STATIONARY - the data enters
    the PE through the 1-col/cycle ldweights path), rhs = vT[:, t, b]
    [128, 1].  Out is sc_ps[:, b:b+1]: scores land directly in [l, b]
    layout in one PSUM bank - no diagonal extraction, no transpose.
  - tail: + c (computed as [1,32], partition-broadcast add), then
    max / exp-accum / ln / sub along the free dim, one 16-KB out DMA.
"""

import os
import sys

sys.path.insert(0, "/opt/trn_rl_repo")

import numpy as np

L = 1024
B = 32
D = 2048
NCORES = 8
L_LOC = L // NCORES          # 128 timesteps per core
NCH = D // 128               # 16 d-chunks
LB = B * L_LOC               # 4096 enc columns per core
G = 2                        # d-chunks per enc DMA (2-MB tiles)
N_TILES = NCH // G           # 8 enc DMAs

_CACHE: dict = {}
last_results = None          # BassKernelResults from the most recent run


def _split_drain_waits(nc):
    """Walrus rejects Drain instructions carrying many sync waits ("Too many
    sync wait commands").  Tile's kernel-tail drain waits on every live
    semaphore lane at once; split it into a chain of single-wait drains."""
    import concourse.mybir as mybir

    for bb in nc.main_func.blocks:
        idx = 0
        while idx < len(bb.instructions):
            inst = bb.instructions[idx]
            if (
                isinstance(inst, mybir.InstDrain)
                and inst.sync_info is not None
                and len(inst.sync_info.on_wait or []) > 1
            ):
                waits = list(inst.sync_info.on_wait)
                spill, keep = waits[:-1], waits[-1:]
                new_insts = []
                for j, w in enumerate(spill):
                    x = mybir.InstDrain(name=f"{inst.name}_w{j}", ins=[], outs=[])
                    x.engine = inst.engine
                    x.sync_info = mybir.SyncInfo(on_wait=[w], on_update=[])
                    x.debug = inst.debug
                    nc.register_instruction(x)
                    new_insts.append(x)
                inst.sync_info = mybir.SyncInfo(
                    on_wait=keep, on_update=list(inst.sync_info.on_update or [])
                )
                bb.instructions[idx:idx] = new_insts
                idx += len(new_insts)
            idx += 1


def build_program():
    """Build (once) the SPMD Bass program shared by all 8 cores."""
    if "nc" in _CACHE:
        return _CACHE["nc"]

    import concourse.bacc as bacc
    import concourse.mybir as mybir
    import concourse.tile as tile

    f32 = mybir.dt.float32
    bf16 = mybir.dt.bfloat16
    Alu = mybir.AluOpType
    Act = mybir.ActivationFunctionType

    nc = bacc.Bacc(
        "TRN2", target_bir_lowering=False, debug=False, num_devices=NCORES
    )

    # encT[d, b, l] = enc[l, b, d] (host-transposed, bf16)
    enct = nc.dram_tensor("enct", [D, LB], bf16, kind="ExternalInput").ap()
    # hbtt[p, 33c + j] = [hidden; b]^T[128c + p, j]
    hbtt = nc.dram_tensor(
        "hbtt", [128, NCH * (B + 1)], bf16, kind="ExternalInput"
    ).ap()
    # wct[t][p, 128c + j] = W[128c + p, 128t + j]  (column-block packed)
    wct = nc.dram_tensor("wct", [NCH, 128, D], bf16, kind="ExternalInput").ap()
    out = nc.dram_tensor("out", [L_LOC, B], f32, kind="ExternalOutput").ap()
    ones = nc.inline_tensor(np.ones((1, 128), dtype=np.float32), "ones").ap()

    with tile.TileContext(nc) as tc:
        with (
            tc.tile_pool(name="pers", bufs=1) as pers,
            tc.tile_pool(name="encp", bufs=4) as encp,
            tc.tile_pool(name="wp", bufs=6) as wp,
            tc.tile_pool(name="psp", bufs=1, space="PSUM") as psp,
        ):
            hbt_sb = pers.tile([128, NCH * (B + 1)], bf16)
            nc.sync.dma_start(hbt_sb[:, :], hbtt[:, :])
            ones_sb = pers.tile([1, 128], f32)
            nc.sync.dma_start(ones_sb[:, :], ones[:, :])

            # PE warm-up: back-to-back matmuls flip the HAM clock gate from
            # 1.2 to 2.4 GHz before the streaming matmuls begin.
            warm_ps = psp.tile([128, 512], f32)
            for i in range(10):
                nc.tensor.matmul(
                    warm_ps[:, :], hbt_sb[:, 0:128], hbt_sb[:, 0:512]
                )
            warm_junk = pers.tile([1, 1], f32)
            nc.vector.tensor_copy(warm_junk[:, :], warm_ps[0:1, 0:1])

            # ---- phase 1: vT[d, b] and c[b] on the PE --------------------
            # vT d-chunk t accumulates over the 16 e-chunks of W col-block t.
            vt_ps = psp.tile([128, NCH, B], f32, tag="vt")
            c_ps = psp.tile([1, B], f32, tag="cp")
            last_w_dma = None
            for t in range(NCH):
                wc = wp.tile([128, D], bf16, tag="wc")
                last_w_dma = nc.sync.dma_start(wc[:, :], wct[t, :, :])
                for c in range(NCH):
                    nc.tensor.matmul(
                        vt_ps[:, t, :],
                        wc[:, 128 * c : 128 * (c + 1)],
                        hbt_sb[:, (B + 1) * c : (B + 1) * c + B],
                        start=(c == 0),
                        stop=(c == NCH - 1),
                        skip_group_check=True,
                    )
                # keep the PE clock hot through the W window (idle PE decays
                # to the 1.2-GHz p-state; junk matmuls hold the gate open)
                for j in range(2):
                    nc.tensor.matmul(
                        warm_ps[:, :], hbt_sb[:, 0:128], hbt_sb[:, 0:512]
                    )
            for c in range(NCH):
                nc.tensor.matmul(
                    c_ps[:, :],
                    hbt_sb[:, (B + 1) * c + B : (B + 1) * (c + 1)],
                    hbt_sb[:, (B + 1) * c : (B + 1) * c + B],
                    start=(c == 0),
                    stop=(c == NCH - 1),
                )
            vt_sb = pers.tile([128, NCH, B], bf16)
            nc.vector.tensor_copy(vt_sb[:, :, :], vt_ps[:, :, :])
            c_sb = pers.tile([1, B], f32)
            nc.vector.tensor_copy(c_sb[:, :], c_ps[:, :])
            # preload the ACT Exp/Ln tables now so the softmax tail does
            # not stall on ACT_TABLE_LOAD
            junk1 = pers.tile([1, 1], f32)
            nc.scalar.activation(junk1[:, :], ones_sb[0:1, 0:1], Act.Exp)
            nc.scalar.activation(junk1[:, :], ones_sb[0:1, 0:1], Act.Ln)

            # ---- phase 2: stream encT, scores on the PE ------------------
            # sc[l, b] accumulates over all 16 d-chunks; each b has its own
            # PSUM column, all in one bank.
            score_ps = psp.tile([L_LOC, B], f32, tag="sc")
            for tt in range(N_TILES):
                et = encp.tile([128, G, LB], bf16, tag="et")
                enc_dma = nc.sync.dma_start(
                    et[:, :, :],
                    enct[128 * G * tt : 128 * G * (tt + 1), :].rearrange(
                        "(g p) x -> p g x", p=128
                    ),
                )
                tile.add_dep_helper(
                    enc_dma.ins,
                    last_w_dma.ins,
                    sync=False,
                    reason="enc stream after W (W-first DMA ordering)",
                )
                for g in range(G):
                    t = G * tt + g
                    for b in range(B):
                        nc.tensor.matmul(
                            score_ps[:, b : b + 1],
                            et[:, g, 128 * b : 128 * (b + 1)],
                            vt_sb[:, t, b : b + 1],
                            start=(t == 0 and b == 0),
                            stop=False,
                            skip_group_check=True,
                        )

            # ---- phase 3: +c via one rank-1 accumulate, then log-softmax -
            # sc[l, b] += ones[l] * c[b] closes every accumulation chain.
            nc.tensor.matmul(
                score_ps[:, :],
                ones_sb[:, :],
                c_sb[:, :],
                start=False,
                stop=True,
                skip_group_check=True,
            )
            m = pers.tile([L_LOC, 1], f32)
            nc.vector.tensor_reduce(
                m[:, :], score_ps[:, :], axis=mybir.AxisListType.X, op=Alu.max
            )
            sm = pers.tile([L_LOC, B], f32)
            nc.vector.tensor_scalar_sub(sm[:, :], score_ps[:, :], m[:, 0:1])
            es = pers.tile([L_LOC, B], f32)
            s1 = pers.tile([L_LOC, 1], f32)
            nc.scalar.activation(
                es[:, :], sm[:, :], Act.Exp, accum_out=s1[:, :]
            )
            ls = pers.tile([L_LOC, 1], f32)
            nc.scalar.activation(ls[:, :], s1[:, :], Act.Ln)
            o = pers.tile([L_LOC, B], f32)
            nc.vector.tensor_scalar_sub(o[:, :], sm[:, :], ls[:, 0:1])
            nc.sync.dma_start(out[:, :], o[:, :])

    nc.compile()
    _split_drain_waits(nc)
    _CACHE["nc"] = nc
    return nc


def make_in_maps(hidden, encoder_outputs, W, b):
    import ml_dtypes

    bf = ml_dtypes.bfloat16
    hidden = np.asarray(hidden, dtype=np.float32)
    enc = np.asarray(encoder_outputs, dtype=np.float32)
    W_ = np.asarray(W, dtype=np.float32)
    b_ = np.asarray(b, dtype=np.float32)

    hb = np.concatenate([hidden, b_[None, :]], axis=0)  # [33, D]
    # hbtt[p, 33c + j] = hb[j, 128c + p]
    hbtt = np.ascontiguousarray(
        hb.T.reshape(NCH, 128, B + 1).transpose(1, 0, 2).reshape(128, NCH * (B + 1))
    ).astype(bf)
    # wct[t, p, 128c + j] = W[128c + p, 128t + j]
    wct = np.ascontiguousarray(
        W_.astype(bf).reshape(NCH, 128, NCH, 128).transpose(2, 1, 0, 3).reshape(
            NCH, 128, D
        )
    )
    # per-core encT[d, b, l] = enc[k*L_LOC + l, b, d]
    enc_bf = enc.astype(bf)  # [L, B, D]
    in_maps = []
    for k in range(NCORES):
        chunk = enc_bf[k * L_LOC : (k + 1) * L_LOC]          # [L_LOC, B, D]
        enct = np.ascontiguousarray(chunk.transpose(2, 1, 0)).reshape(D, LB)
        in_maps.append({"enct": enct, "hbtt": hbtt, "wct": wct})
    return in_maps


def kernel(hidden, encoder_outputs, W, b):
    """Full inputs in, full [1, 1, L, B] output out; runs on 8 NeuronCores."""
    global last_results
    from concourse.bass_utils import run_bass_kernel_spmd

    nc = build_program()
    in_maps = make_in_maps(hidden, encoder_outputs, W, b)
    res = run_bass_kernel_spmd(
        nc,
        in_maps,
        list(range(NCORES)),
        trace=bool(os.environ.get("KERNEL_TRACE")),
    )
    last_results = res
    chunks = [res.results[k]["out"] for k in range(NCORES)]
    full = np.concatenate(chunks, axis=0).reshape(1, 1, L, B)
    return full.astype(np.float32)
